# revision 25
# baseline (speedup 1.0000x reference)
"""Trainium2 Bass kernel for nn_ExtractorMLP (GNN edge cosine-similarity logits).

Math: out[e] = cos(MLP(emb[col[e]]), MLP(emb[row[e]])) for E edges, where
MLP(x) = relu(x @ W1.T + b1) @ W2.T + b2, cos uses torch eps=1e-8 semantics.

Strategy (8 cores, SPMD, identical program, per-core edge shards):
  Phase 1 (replicated): run the MLP over ALL N nodes once per core
    (12x fewer FLOPs than per-edge MLP), normalize each output row,
    store a bf16 table in core-local DRAM, split into gn_lo (nodes
    < 32768) and gn_hi so the low half's gathers can overlap the
    second half of phase 1.
    Engine balance per 512-node block: PE 12 matmuls (b2 skipped when
    zero), ACT relu x2 + 2 square+accum norm chunks + sqrt, DVE
    psum->bf16 copy + 2 STT norm chunks + normalize multiply.
  Phase 2 (edge shard, E/8 per core): chunked dma_gather of table rows
    for col/row endpoints (int16 indices, edges pre-grouped host-side
    into 4 groups by (col<32768, row<32768)), then a single fused DVE
    multiply-cumsum instruction per chunk; per-edge dots fall out as
    differences of page-boundary prefix values.  Group (lo,lo) is
    issued as soon as gn_lo is fully written.
"""

import re
import sys

for _p in ("/opt/trn_rl_repo",):
    if _p not in sys.path:
        sys.path.insert(0, _p)

import numpy as np
import ml_dtypes

import concourse.bass as bass
import concourse.bacc as bacc
import concourse.mybir as mybir
import concourse.tile as tile
from concourse.tile import add_dep_helper
from concourse.bass_utils import run_bass_kernel_spmd

BF16 = mybir.dt.bfloat16
F32 = mybir.dt.float32
I16 = mybir.dt.int16

# Problem sizes (hardcoded per harness contract)
N, H, E = 50000, 256, 300000
NCORES = 8
F = 512                          # node-phase free-dim block (nodes per block)
NPAD = ((N + F - 1) // F) * F    # 50176
NBLK = NPAD // F                 # 98
EPC = E // NCORES                # 37500 edges per core
HALF = 32768                     # int16 index range split point
BLO = HALF // F                  # 64 blocks in the low table half
GCHUNK = 2048                    # edges per dma_gather
EPS = 1e-8


def _group_caps(epc):
    """Compile-time per-group capacities: mean + 8 sigma, rounded to 1024
    (so every gather chunk is 2048 or 1024 edges -> only two tile shapes)."""
    p = HALF / N
    probs = [p * p, p * (1 - p), (1 - p) * p, (1 - p) * (1 - p)]
    caps = []
    for pr in probs:
        mean = epc * pr
        sig = (epc * pr * (1 - pr)) ** 0.5
        caps.append(int(np.ceil((mean + 8 * sig) / 1024)) * 1024)
    return caps


GCAPS = _group_caps(EPC)         # [16896, 9216, 9216, 5120]
GOFFS = [int(x) for x in np.cumsum([0] + GCAPS[:-1])]
TOTE = sum(GCAPS)                # 40448


# ---- custom DVE op: out = cumsum(in0 * in1) along the free stream ----
_CUMSUM_OP = None


def _get_mult_cumsum_op():
    global _CUMSUM_OP
    if _CUMSUM_OP is not None:
        return _CUMSUM_OP
    from concourse.dve_ops import OPS, DveOp
    from concourse.dve_spec import Spec, Src0, Src1, scan, AluOp

    for op in OPS:
        if op.name == "TT_MULT_CUMSUM_ANT":
            _CUMSUM_OP = op
            return op
    spec = Spec(
        body=scan(AluOp.ADD, Src0 * Src1),
        reference=lambda in0, in1, s0, s1, imm2: np.cumsum(
            in0.astype(np.float32) * in1.astype(np.float32), axis=-1
        ),
    )
    op = DveOp("TT_MULT_CUMSUM_ANT", spec, subdim=False, uops_sha={})
    OPS.append(op)
    from concourse import dve_ops as _dv
    _dv.CUSTOM_DVE_SPECS[op.name] = op.spec
    _dv._SUB_OPCODE_FOR_NAME[op.name] = _dv._CUSTOM_DVE_ROW_BASE + len(OPS) - 1
    assert max(_dv._SUB_OPCODE_FOR_NAME.values()) < 0x20
    for ver in ("v3", "v4"):
        try:
            op.compile(ver)
        except ValueError as e:
            m = re.search(r'"(?:v\d)"\]="([0-9a-f]+)"', str(e))
            if not m:
                raise
            op.uops_sha[ver] = m.group(1)
            op.compile(ver)  # re-validate
    _CUMSUM_OP = op
    return op


def build_bass(n_pad, n_blk, f, gcaps, half, gchunk, add_b2=False, table_dt=BF16):
    """Build the SPMD Bass module."""
    nc = bacc.Bacc("TRN2", target_bir_lowering=False, num_swdge_queues=4)
    h = H
    tote = sum(gcaps)
    goffs = [int(x) for x in np.cumsum([0] + list(gcaps[:-1]))]
    blo = min(BLO, n_blk)
    n_lo = blo * f
    cumsum_op = _get_mult_cumsum_op()

    embT = nc.dram_tensor("embT", [h, n_pad], BF16, kind="ExternalInput")
    w1t = nc.dram_tensor("w1t", [h, h], BF16, kind="ExternalInput")
    w2t = nc.dram_tensor("w2t", [h, h], BF16, kind="ExternalInput")
    b1c = nc.dram_tensor("b1c", [h, 1], F32, kind="ExternalInput")
    if add_b2:
        b2quad = nc.dram_tensor("b2quad", [1, 4 * h], BF16, kind="ExternalInput")
    colw = nc.dram_tensor("colw", [128, tote // 16], I16, kind="ExternalInput")
    roww = nc.dram_tensor("roww", [128, tote // 16], I16, kind="ExternalInput")
    dots_out = nc.dram_tensor("dots", [128, tote // 128], F32, kind="ExternalOutput")
    gn_lo = nc.dram_tensor("gn_lo", [n_lo, h], table_dt)       # internal
    gn_hi = nc.dram_tensor("gn_hi", [n_pad - n_lo, h], table_dt)

    AF = mybir.ActivationFunctionType
    OP = mybir.AluOpType
    AX = mybir.AxisListType

    with tile.TileContext(nc) as tc:
        with (
            tc.tile_pool(name="const", bufs=1) as constp,
            tc.tile_pool(name="xt", bufs=3) as xtp,
            tc.tile_pool(name="h1", bufs=3) as h1p,
            tc.tile_pool(name="gg", bufs=3) as gp,
            tc.tile_pool(name="jk", bufs=2) as jp,
            tc.tile_pool(name="small", bufs=4) as sp,
            tc.tile_pool(name="ps1", bufs=4, space="PSUM") as ps1,
            tc.tile_pool(name="ps2", bufs=2, space="PSUM") as ps2,
            tc.tile_pool(name="ebuf16", bufs=4) as ep16,
            tc.tile_pool(name="ebuf8", bufs=2) as ep8,
            tc.tile_pool(name="scan", bufs=2) as scanp,
            tc.tile_pool(name="ext", bufs=2) as extp,
        ):
            # ---- constants ----
            w1k = []
            w2k = []
            b1t = []
            for k in range(2):
                t_ = constp.tile([128, h], BF16, tag=f"w1_{k}")
                nc.sync.dma_start(out=t_[:], in_=w1t[k * 128:(k + 1) * 128, :])
                w1k.append(t_)
                t_ = constp.tile([128, h], BF16, tag=f"w2_{k}")
                nc.sync.dma_start(out=t_[:], in_=w2t[k * 128:(k + 1) * 128, :])
                w2k.append(t_)
                t_ = constp.tile([128, 1], F32, tag=f"b1_{k}")
                nc.sync.dma_start(out=t_[:], in_=b1c[k * 128:(k + 1) * 128, :])
                b1t.append(t_)
            if add_b2:
                b2q = constp.tile([1, 4 * h], BF16, tag="b2q")
                nc.sync.dma_start(out=b2q[:], in_=b2quad[:])
                ones_row = constp.tile([1, 128], BF16, tag="ones_row")
                nc.vector.memset(ones_row[:], 1.0)
            eps2 = constp.tile([128, 1], F32, tag="eps2")
            nc.vector.memset(eps2[:], EPS * EPS)
            colsb = constp.tile([128, tote // 16], I16, tag="colsb")
            nc.sync.dma_start(out=colsb[:], in_=colw[:])
            rowsb = constp.tile([128, tote // 16], I16, tag="rowsb")
            nc.sync.dma_start(out=rowsb[:], in_=roww[:])
            dots = constp.tile([128, tote // 128], F32, tag="dots")

            lo_writes = []
            hi_writes = []

            # ---- phase 2: gather emission (Pool queue) is decoupled from
            # dot-compute emission (DVE/ACT queues).  Engine queues are FIFO
            # and a wait blocks the queue head, so a chunk's cumsum must be
            # emitted well after its gathers were issued. ----
            state = {"qi": 0, "prev": None}
            bases = [(0, 0), (0, half), (half, 0), (half, half)]

            def chunk_list(g):
                out = []
                for c0 in range(0, gcaps[g], gchunk):
                    out.append((g, c0, min(gchunk, gcaps[g] - c0)))
                return out

            def emit_gathers(g, c0, nI, dep_writes):
                cb, rb = bases[g]
                src_c = (gn_hi if (cb and n_pad > n_lo) else gn_lo)[:]
                src_r = (gn_hi if (rb and n_pad > n_lo) else gn_lo)[:]
                nb = nI // 128
                w0 = (goffs[g] + c0) // 16
                ep = ep16 if nb == 16 else ep8
                g1 = ep.tile([128, nb, h], table_dt, tag=f"g1_{nb}")
                g2 = ep.tile([128, nb, h], table_dt, tag=f"g2_{nb}")
                gi1 = nc.gpsimd.dma_gather(
                    g1[:], src_c, colsb[:, w0:w0 + nI // 16],
                    nI, nI, h, transpose=False, single_packet=False,
                    queue_num=state["qi"] % 4,
                )
                state["qi"] += 1
                gi2 = nc.gpsimd.dma_gather(
                    g2[:], src_r, rowsb[:, w0:w0 + nI // 16],
                    nI, nI, h, transpose=False, single_packet=False,
                    queue_num=state["qi"] % 4,
                )
                state["qi"] += 1
                if dep_writes:
                    # first gather of a segment: true sync deps on the table
                    # writes it may read (tile does not track the gather's
                    # dram input); later gathers inherit via in-order engine
                    # dispatch
                    for wins in dep_writes:
                        add_dep_helper(gi1.ins, wins.ins, sync=True,
                                       reason="table writes before gather")
                if state["prev"] is not None:
                    add_dep_helper(gi1.ins, state["prev"].ins, sync=False,
                                   reason="gather order")
                add_dep_helper(gi2.ins, gi1.ins, sync=False,
                               reason="gather order")
                state["prev"] = gi2
                return g1, g2

            def emit_compute(g, c0, nI, g1, g2):
                # fused dot: cumsum(g1*g2); per-edge dots are diffs of
                # page-final prefix values
                nb = nI // 128
                sc = scanp.tile([128, nb, h], F32, tag=f"sc_{nb}")
                nc.vector._custom_dve(
                    cumsum_op, out=sc[:], in0=g1[:], in1=g2[:]
                )
                ext = extp.tile([128, nb], F32, tag=f"ext_{nb}")
                nc.scalar.activation(
                    out=ext[:],
                    in_=sc[:, :, h - 1:h].rearrange("p n o -> p (n o)"),
                    func=AF.Copy,
                )
                q0 = (goffs[g] + c0) // 128
                nc.scalar.activation(
                    out=dots[:, q0:q0 + 1], in_=ext[:, 0:1], func=AF.Copy)
                if nb > 1:
                    nc.vector.tensor_tensor(
                        out=dots[:, q0 + 1:q0 + nb],
                        in0=ext[:, 1:nb], in1=ext[:, 0:nb - 1],
                        op=OP.subtract,
                    )

            # group-0 gathers are all issued right after block blo-1 (Pool
            # runs them serially while phase 1 continues); their dot-computes
            # are paced ~6 blocks apart so each cumsum's gather has landed
            # before it reaches the DVE queue head
            g0_chunks = chunk_list(0)
            gathered = {}          # (g, c0) -> (g1, g2)
            compute_at = {b_: [] for b_ in range(n_blk)}
            if n_blk > blo:
                for i, ch in enumerate(g0_chunks):
                    b_ = blo + 4 + 5 * i
                    if b_ < n_blk:
                        compute_at[b_].append(ch)

            done_compute = set()

            # ---- phase 1: node MLP -> normalized bf16 table ----
            for b in range(n_blk):
                n0 = b * f
                xt = xtp.tile([128, 2, f], BF16, tag="xt")
                nc.sync.dma_start(
                    out=xt[:],
                    in_=embT[:, n0:n0 + f].rearrange("(c p) n -> p c n", p=128),
                )
                ht = []
                for t in range(2):
                    p1 = ps1.tile([128, f], F32, tag="p1")
                    for k in range(2):
                        nc.tensor.matmul(
                            p1[:],
                            lhsT=w1k[k][:, t * 128:(t + 1) * 128],
                            rhs=xt[:, k, :],
                            start=(k == 0),
                            stop=(k == 1),
                        )
                    h_ = h1p.tile([128, f], BF16, tag=f"h1_{t}")
                    if add_b2:
                        nc.scalar.activation(
                            h_[:], p1[:], AF.Relu, bias=b1t[t][:])
                    elif t == 1 and b % 2 == 0:
                        # zero-bias fast path: alternate one relu to DVE to
                        # balance ACT/DVE load
                        nc.vector.tensor_scalar_max(h_[:], p1[:], 0.0)
                    else:
                        nc.scalar.activation(h_[:], p1[:], AF.Relu)
                    ht.append(h_)
                nch = f // 128
                p2 = ps2.tile([128, nch, h], F32, tag="p2")
                for c in range(nch):
                    nc.tensor.matmul(
                        p2[:, c, :], lhsT=ht[0][:, c * 128:(c + 1) * 128],
                        rhs=w2k[0][:], start=True, stop=False,
                    )
                    nc.tensor.matmul(
                        p2[:, c, :], lhsT=ht[1][:, c * 128:(c + 1) * 128],
                        rhs=w2k[1][:], start=False, stop=not add_b2,
                    )
                    if add_b2:
                        nc.tensor.matmul(
                            p2[:, c, :], lhsT=ones_row[:],
                            rhs=b2q[:, c * h:(c + 1) * h],
                            start=False, stop=True,
                        )
                # norms^2: ACT squares psum into bf16; reduce on Pool while
                # it is idle (before the gathers start), on DVE afterwards
                sqb = gp.tile([128, nch, h], BF16, tag="sqb")
                nc.scalar.activation(sqb[:], p2[:], AF.Square)
                n2 = sp.tile([128, nch], F32, tag="n2")
                nc.vector.tensor_reduce(
                    out=n2[:], in_=sqb[:], axis=AX.X, op=OP.add)
                # max(sqrt(n2), eps) == sqrt(n2 + eps^2) here (exact for
                # n2=0 and for any real row)
                s_ = sp.tile([128, nch], F32, tag="s")
                nc.scalar.activation(s_[:], n2[:], AF.Sqrt, bias=eps2[:])
                invb = sp.tile([128, nch], BF16, tag="invb")
                with nc.allow_low_precision(reason="bf16 inv-norm scale"):
                    nc.vector.reciprocal(invb[:], s_[:])
                gnb = gp.tile([128, nch, h], table_dt, tag="gnb")
                nc.vector.tensor_tensor(
                    out=gnb[:], in0=p2[:],
                    in1=invb[:].to_broadcast([128, nch, h]), op=OP.mult,
                )
                if b < blo:
                    dst = gn_lo[n0:n0 + f, :]
                else:
                    dst = gn_hi[n0 - n_lo:n0 - n_lo + f, :]
                wi = nc.sync.dma_start(
                    out=dst.rearrange("(c p) h -> p c h", p=128),
                    in_=gnb[:],
                )
                (lo_writes if b < blo else hi_writes).append(wi)

                if b == blo - 1 and n_blk > blo:
                    for i, (g_, c0_, nI_) in enumerate(g0_chunks):
                        gathered[(g_, c0_)] = emit_gathers(
                            g_, c0_, nI_, list(lo_writes) if i == 0 else [])
                for g_, c0_, nI_ in compute_at[b]:
                    emit_compute(g_, c0_, nI_, *gathered[(g_, c0_)])
                    done_compute.add((g_, c0_))

            # remaining gathers (Pool-serial; WAR on the tile rings paces
            # them against the computes), then remaining computes in order
            rest_computes = [c for c in g0_chunks
                             if (c[0], c[1]) not in done_compute]
            if n_blk <= blo:
                for i, (g_, c0_, nI_) in enumerate(g0_chunks):
                    gathered[(g_, c0_)] = emit_gathers(
                        g_, c0_, nI_, list(lo_writes) if i == 0 else [])
            for g in (1, 2, 3):
                for i, (g_, c0_, nI_) in enumerate(chunk_list(g)):
                    deps = []
                    if g == 1 and i == 0:
                        deps = list(hi_writes)
                        if n_blk <= blo:
                            deps += list(lo_writes)
                    gathered[(g_, c0_)] = emit_gathers(g_, c0_, nI_, deps)
                    rest_computes.append((g_, c0_, nI_))
            for g_, c0_, nI_ in rest_computes:
                emit_compute(g_, c0_, nI_, *gathered[(g_, c0_)])
            nc.sync.dma_start(out=dots_out[:], in_=dots[:])

    return nc


def make_inputs(emb, W1, b1, W2, b2, col, row, n_pad, gcaps, ncores):
    """Host-side prep: transposes, bf16 rounding, per-core group shards.

    Returns (in_maps, scatter) where scatter[c] = (positions, lens)
    for reassembling per-core outputs.
    """
    h = emb.shape[1]
    embT = np.zeros((h, n_pad), dtype=ml_dtypes.bfloat16)
    embT[:, :emb.shape[0]] = emb.astype(ml_dtypes.bfloat16).T
    w1t = np.ascontiguousarray(W1.astype(ml_dtypes.bfloat16).T)
    w2t = np.ascontiguousarray(W2.astype(ml_dtypes.bfloat16).T)
    b1c = np.ascontiguousarray(b1.astype(np.float32).reshape(h, 1))
    b2quad = np.tile(b2.astype(ml_dtypes.bfloat16).reshape(1, h), (1, 4))
    epc = len(col) // ncores
    goffs = [int(x) for x in np.cumsum([0] + list(gcaps[:-1]))]
    tote = sum(gcaps)

    def wrap16(a):
        return np.tile(a.reshape(-1, 16).T, (8, 1)).astype(np.int16)

    in_maps = []
    scatter = []
    for c in range(ncores):
        cs = col[c * epc:(c + 1) * epc].astype(np.int64)
        rs = row[c * epc:(c + 1) * epc].astype(np.int64)
        gid = (cs >= HALF) * 2 + (rs >= HALF)
        # padded tail slots get -1: dma_gather trims trailing negatives, so
        # pad slots cost no descriptor generation and no transfer
        colw = np.zeros(tote, dtype=np.int16)
        roww = np.zeros(tote, dtype=np.int16)
        positions = []
        lens = []
        for g in range(4):
            pos = np.nonzero(gid == g)[0]
            pos = pos[np.argsort(cs[pos], kind="stable")]
            ng = len(pos)
            assert ng <= gcaps[g], f"group {g} overflow: {ng} > {gcaps[g]}"
            cb = HALF if g >= 2 else 0
            rb = HALF if g % 2 else 0
            colw[goffs[g]:goffs[g] + ng] = (cs[pos] - cb).astype(np.int16)
            roww[goffs[g]:goffs[g] + ng] = (rs[pos] - rb).astype(np.int16)
            positions.append(pos)
            lens.append(ng)
        in_maps.append({
            "embT": embT, "w1t": w1t, "w2t": w2t, "b1c": b1c,
            "b2quad": b2quad,
            "colw": wrap16(colw), "roww": wrap16(roww),
        })
        scatter.append((positions, lens))
    return in_maps, scatter


def unshard_output(outs, scatter, gcaps, epc, ncores):
    goffs = [int(x) for x in np.cumsum([0] + list(gcaps[:-1]))]
    parts = []
    for c in range(ncores):
        dots = np.asarray(outs[c]["dots"]).T.reshape(-1)
        positions, lens = scatter[c]
        res = np.empty(epc, dtype=np.float32)
        for g in range(4):
            res[positions[g]] = dots[goffs[g]:goffs[g] + lens[g]]
        parts.append(res)
    return np.concatenate(parts)


_NC_CACHE = {}


def get_nc(add_b2=False):
    key = ("nc", bool(add_b2))
    if key not in _NC_CACHE:
        nc_ = build_bass(NPAD, NBLK, F, GCAPS, HALF, GCHUNK, add_b2=add_b2)
        nc_.compile()
        _NC_CACHE[key] = nc_
    return _NC_CACHE[key]


def kernel(emb, edge_index, W1, b1, W2, b2):
    emb = np.asarray(emb)
    edge_index = np.asarray(edge_index)
    W1, b1, W2, b2 = (np.asarray(a) for a in (W1, b1, W2, b2))
    col = edge_index[0].astype(np.int64)
    row = edge_index[1].astype(np.int64)

    nc = get_nc(add_b2=bool(np.any(b2)))
    in_maps, scatter = make_inputs(emb, W1, b1, W2, b2, col, row, NPAD, GCAPS, NCORES)
    # drop inputs the chosen program does not declare
    declared = {a.memorylocations[0].name
                for a_ in [nc.m.functions[0].allocations] for a in a_
                if isinstance(a, mybir.MemoryLocationSet)
                and a.kind == "ExternalInput"}
    in_maps = [{k: v for k, v in m.items() if k in declared} for m in in_maps]
    res = run_bass_kernel_spmd(nc, in_maps, core_ids=list(range(NCORES)))
    return unshard_output(res.results, scatter, GCAPS, EPC, NCORES).astype(np.float32)


# revision 28
# speedup vs baseline: 1.0404x; 1.0404x over previous
"""Trainium2 Bass kernel for nn_ExtractorMLP (GNN edge cosine-similarity logits).

Math: out[e] = cos(MLP(emb[col[e]]), MLP(emb[row[e]])) for E edges, where
MLP(x) = relu(x @ W1.T + b1) @ W2.T + b2, cos uses torch eps=1e-8 semantics.

Strategy (8 cores, SPMD, identical program, per-core edge shards):
  Phase 1 (replicated): run the MLP over ALL N nodes once per core
    (12x fewer FLOPs than per-edge MLP), normalize each output row,
    store a bf16 table in core-local DRAM, split into gn_lo (nodes
    < 32768) and gn_hi so the low half's gathers can overlap the
    second half of phase 1.
    Engine balance per 512-node block: PE 12 matmuls (b2 skipped when
    zero), ACT relu x2 + 2 square+accum norm chunks + sqrt, DVE
    psum->bf16 copy + 2 STT norm chunks + normalize multiply.
  Phase 2 (edge shard, E/8 per core): chunked dma_gather of table rows
    for col/row endpoints (int16 indices, edges pre-grouped host-side
    into 4 groups by (col<32768, row<32768)), then a single fused DVE
    multiply-cumsum instruction per chunk; per-edge dots fall out as
    differences of page-boundary prefix values.  Group (lo,lo) is
    issued as soon as gn_lo is fully written.
"""

import re
import sys

for _p in ("/opt/trn_rl_repo",):
    if _p not in sys.path:
        sys.path.insert(0, _p)

import numpy as np
import ml_dtypes

import concourse.bass as bass
import concourse.bacc as bacc
import concourse.mybir as mybir
import concourse.tile as tile
from concourse.tile import add_dep_helper
from concourse.bass_utils import run_bass_kernel_spmd

BF16 = mybir.dt.bfloat16
F32 = mybir.dt.float32
I16 = mybir.dt.int16

# Problem sizes (hardcoded per harness contract)
N, H, E = 50000, 256, 300000
NCORES = 8
F = 512                          # node-phase free-dim block (nodes per block)
NPAD = ((N + F - 1) // F) * F    # 50176
NBLK = NPAD // F                 # 98
EPC = E // NCORES                # 37500 edges per core
HALF = 32768                     # int16 index range split point
BLO = HALF // F                  # 64 blocks in the low table half
GCHUNK = 2048                    # edges per dma_gather
EPS = 1e-8


def _group_caps(epc):
    """Compile-time per-group capacities: mean + 8 sigma, rounded to 1024
    (so every gather chunk is 2048 or 1024 edges -> only two tile shapes)."""
    p = HALF / N
    probs = [p * p, p * (1 - p), (1 - p) * p, (1 - p) * (1 - p)]
    caps = []
    for pr in probs:
        mean = epc * pr
        sig = (epc * pr * (1 - pr)) ** 0.5
        caps.append(int(np.ceil((mean + 8 * sig) / 1024)) * 1024)
    return caps


GCAPS = _group_caps(EPC)         # [17408, 9216, 9216, 5120]
GOFFS = [int(x) for x in np.cumsum([0] + GCAPS[:-1])]
TOTE = sum(GCAPS)                # 40960


def _g0_bounds():
    """Static node bounds for group-0 chunks sorted by max(col,row).

    Chunk k (slots [k*GCHUNK, (k+1)*GCHUNK)) only references table rows
    < B_k, where B_k is chosen so that P(#edges with max(col,row) < B_k
    across EPC draws < (k+1)*GCHUNK) is ~1e-9 (6.5-sigma binomial bound:
    #edges below B ~ Binom(EPC, (B/N)^2)).  Lets chunk k's gathers start
    once table blocks 0..ceil(B_k/F)-1 are written, well before the whole
    low half is done.
    """
    bounds = []
    for k in range(GCAPS[0] // GCHUNK + 1):
        need = min((k + 1) * GCHUNK, GCAPS[0])
        u = (6.5 + np.sqrt(6.5 * 6.5 + 4.0 * need)) / 2.0
        b = int(np.ceil(N * u / np.sqrt(EPC)))
        bounds.append(min(b, HALF))
    return bounds


G0_BOUNDS = _g0_bounds()


# ---- custom DVE op: out = cumsum(in0 * in1) along the free stream ----
_CUMSUM_OP = None


def _get_mult_cumsum_op():
    global _CUMSUM_OP
    if _CUMSUM_OP is not None:
        return _CUMSUM_OP
    from concourse.dve_ops import OPS, DveOp
    from concourse.dve_spec import Spec, Src0, Src1, scan, AluOp

    for op in OPS:
        if op.name == "TT_MULT_CUMSUM_ANT":
            _CUMSUM_OP = op
            return op
    spec = Spec(
        body=scan(AluOp.ADD, Src0 * Src1),
        reference=lambda in0, in1, s0, s1, imm2: np.cumsum(
            in0.astype(np.float32) * in1.astype(np.float32), axis=-1
        ),
    )
    op = DveOp("TT_MULT_CUMSUM_ANT", spec, subdim=False, uops_sha={})
    OPS.append(op)
    from concourse import dve_ops as _dv
    _dv.CUSTOM_DVE_SPECS[op.name] = op.spec
    _dv._SUB_OPCODE_FOR_NAME[op.name] = _dv._CUSTOM_DVE_ROW_BASE + len(OPS) - 1
    assert max(_dv._SUB_OPCODE_FOR_NAME.values()) < 0x20
    for ver in ("v3", "v4"):
        try:
            op.compile(ver)
        except ValueError as e:
            m = re.search(r'"(?:v\d)"\]="([0-9a-f]+)"', str(e))
            if not m:
                raise
            op.uops_sha[ver] = m.group(1)
            op.compile(ver)  # re-validate
    _CUMSUM_OP = op
    return op


def build_bass(n_pad, n_blk, f, gcaps, half, gchunk, add_b2=False, table_dt=BF16):
    """Build the SPMD Bass module."""
    nc = bacc.Bacc("TRN2", target_bir_lowering=False, num_swdge_queues=4)
    h = H
    tote = sum(gcaps)
    goffs = [int(x) for x in np.cumsum([0] + list(gcaps[:-1]))]
    blo = min(BLO, n_blk)
    n_lo = blo * f
    cumsum_op = _get_mult_cumsum_op()

    embT = nc.dram_tensor("embT", [h, n_pad], BF16, kind="ExternalInput")
    w1t = nc.dram_tensor("w1t", [h, h], BF16, kind="ExternalInput")
    w2t = nc.dram_tensor("w2t", [h, h], BF16, kind="ExternalInput")
    b1c = nc.dram_tensor("b1c", [h, 1], F32, kind="ExternalInput")
    if add_b2:
        b2quad = nc.dram_tensor("b2quad", [1, 4 * h], BF16, kind="ExternalInput")
    colw = nc.dram_tensor("colw", [128, tote // 16], I16, kind="ExternalInput")
    roww = nc.dram_tensor("roww", [128, tote // 16], I16, kind="ExternalInput")
    dots_out = nc.dram_tensor("dots", [128, tote // 128], F32, kind="ExternalOutput")
    gn_lo = nc.dram_tensor("gn_lo", [n_lo, h], table_dt)       # internal
    gn_hi = nc.dram_tensor("gn_hi", [n_pad - n_lo, h], table_dt)

    AF = mybir.ActivationFunctionType
    OP = mybir.AluOpType
    AX = mybir.AxisListType

    with tile.TileContext(nc) as tc:
        with (
            tc.tile_pool(name="const", bufs=1) as constp,
            tc.tile_pool(name="xt", bufs=3) as xtp,
            tc.tile_pool(name="h1", bufs=3) as h1p,
            tc.tile_pool(name="gg", bufs=3) as gp,
            tc.tile_pool(name="jk", bufs=2) as jp,
            tc.tile_pool(name="small", bufs=4) as sp,
            tc.tile_pool(name="ps1", bufs=4, space="PSUM") as ps1,
            tc.tile_pool(name="ps2", bufs=2, space="PSUM") as ps2,
            tc.tile_pool(name="ebuf16", bufs=4) as ep16,
            tc.tile_pool(name="ebuf8", bufs=2) as ep8,
            tc.tile_pool(name="scan", bufs=2) as scanp,
            tc.tile_pool(name="ext", bufs=2) as extp,
        ):
            # ---- constants ----
            w1k = []
            w2k = []
            b1t = []
            for k in range(2):
                t_ = constp.tile([128, h], BF16, tag=f"w1_{k}")
                nc.sync.dma_start(out=t_[:], in_=w1t[k * 128:(k + 1) * 128, :])
                w1k.append(t_)
                t_ = constp.tile([128, h], BF16, tag=f"w2_{k}")
                nc.sync.dma_start(out=t_[:], in_=w2t[k * 128:(k + 1) * 128, :])
                w2k.append(t_)
                t_ = constp.tile([128, 1], F32, tag=f"b1_{k}")
                nc.sync.dma_start(out=t_[:], in_=b1c[k * 128:(k + 1) * 128, :])
                b1t.append(t_)
            if add_b2:
                b2q = constp.tile([1, 4 * h], BF16, tag="b2q")
                nc.sync.dma_start(out=b2q[:], in_=b2quad[:])
                ones_row = constp.tile([1, 128], BF16, tag="ones_row")
                nc.vector.memset(ones_row[:], 1.0)
            eps2 = constp.tile([128, 1], F32, tag="eps2")
            nc.vector.memset(eps2[:], EPS * EPS)
            colsb = constp.tile([128, tote // 16], I16, tag="colsb")
            nc.sync.dma_start(out=colsb[:], in_=colw[:])
            rowsb = constp.tile([128, tote // 16], I16, tag="rowsb")
            nc.sync.dma_start(out=rowsb[:], in_=roww[:])
            dots = constp.tile([128, tote // 128], F32, tag="dots")

            lo_writes = []
            hi_writes = []

            # ---- phase 2: gather emission (Pool queue) is decoupled from
            # dot-compute emission (DVE/ACT queues).  Engine queues are FIFO
            # and a wait blocks the queue head, so a chunk's cumsum must be
            # emitted well after its gathers were issued. ----
            state = {"qi": 0, "prev": None}
            bases = [(0, 0), (0, half), (half, 0), (half, half)]

            def chunk_list(g):
                out = []
                for c0 in range(0, gcaps[g], gchunk):
                    out.append((g, c0, min(gchunk, gcaps[g] - c0)))
                return out

            def emit_gathers(g, c0, nI, dep_writes):
                cb, rb = bases[g]
                src_c = (gn_hi if (cb and n_pad > n_lo) else gn_lo)[:]
                src_r = (gn_hi if (rb and n_pad > n_lo) else gn_lo)[:]
                nb = nI // 128
                w0 = (goffs[g] + c0) // 16
                ep = ep16 if nb == 16 else ep8
                g1 = ep.tile([128, nb, h], table_dt, tag=f"g1_{nb}")
                g2 = ep.tile([128, nb, h], table_dt, tag=f"g2_{nb}")
                gi1 = nc.gpsimd.dma_gather(
                    g1[:], src_c, colsb[:, w0:w0 + nI // 16],
                    nI, nI, h, transpose=False, single_packet=False,
                    queue_num=state["qi"] % 4,
                )
                state["qi"] += 1
                gi2 = nc.gpsimd.dma_gather(
                    g2[:], src_r, rowsb[:, w0:w0 + nI // 16],
                    nI, nI, h, transpose=False, single_packet=False,
                    queue_num=state["qi"] % 4,
                )
                state["qi"] += 1
                if dep_writes:
                    # first gather of a segment: true sync deps on the table
                    # writes it may read (tile does not track the gather's
                    # dram input); later gathers inherit via in-order engine
                    # dispatch
                    for wins in dep_writes:
                        add_dep_helper(gi1.ins, wins.ins, sync=True,
                                       reason="table writes before gather")
                if state["prev"] is not None:
                    add_dep_helper(gi1.ins, state["prev"].ins, sync=False,
                                   reason="gather order")
                add_dep_helper(gi2.ins, gi1.ins, sync=False,
                               reason="gather order")
                state["prev"] = gi2
                return g1, g2

            def emit_compute(g, c0, nI, g1, g2):
                # fused dot: cumsum(g1*g2); per-edge dots are diffs of
                # page-final prefix values
                nb = nI // 128
                sc = scanp.tile([128, nb, h], F32, tag=f"sc_{nb}")
                nc.vector._custom_dve(
                    cumsum_op, out=sc[:], in0=g1[:], in1=g2[:]
                )
                ext = extp.tile([128, nb], F32, tag=f"ext_{nb}")
                nc.scalar.activation(
                    out=ext[:],
                    in_=sc[:, :, h - 1:h].rearrange("p n o -> p (n o)"),
                    func=AF.Copy,
                )
                q0 = (goffs[g] + c0) // 128
                nc.scalar.activation(
                    out=dots[:, q0:q0 + 1], in_=ext[:, 0:1], func=AF.Copy)
                if nb > 1:
                    nc.vector.tensor_tensor(
                        out=dots[:, q0 + 1:q0 + nb],
                        in0=ext[:, 1:nb], in1=ext[:, 0:nb - 1],
                        op=OP.subtract,
                    )

            # group-0 gathers are all issued right after block blo-1 (Pool
            # runs them serially while phase 1 continues); their dot-computes
            # are paced ~6 blocks apart so each cumsum's gather has landed
            # before it reaches the DVE queue head
            g0_chunks = chunk_list(0)
            gathered = {}          # (g, c0) -> (g1, g2)
            compute_at = {b_: [] for b_ in range(n_blk)}
            if n_blk > blo:
                for i, ch in enumerate(g0_chunks):
                    b_ = blo + 6 + 6 * i
                    if b_ < n_blk:
                        compute_at[b_].append(ch)

            done_compute = set()

            # ---- phase 1: node MLP -> normalized bf16 table ----
            for b in range(n_blk):
                n0 = b * f
                xt = xtp.tile([128, 2, f], BF16, tag="xt")
                nc.sync.dma_start(
                    out=xt[:],
                    in_=embT[:, n0:n0 + f].rearrange("(c p) n -> p c n", p=128),
                )
                ht = []
                for t in range(2):
                    p1 = ps1.tile([128, f], F32, tag="p1")
                    for k in range(2):
                        nc.tensor.matmul(
                            p1[:],
                            lhsT=w1k[k][:, t * 128:(t + 1) * 128],
                            rhs=xt[:, k, :],
                            start=(k == 0),
                            stop=(k == 1),
                        )
                    h_ = h1p.tile([128, f], BF16, tag=f"h1_{t}")
                    nc.scalar.activation(h_[:], p1[:], AF.Relu, bias=b1t[t][:])
                    ht.append(h_)
                nch = f // 128
                p2 = ps2.tile([128, nch, h], F32, tag="p2")
                for c in range(nch):
                    nc.tensor.matmul(
                        p2[:, c, :], lhsT=ht[0][:, c * 128:(c + 1) * 128],
                        rhs=w2k[0][:], start=True, stop=False,
                    )
                    nc.tensor.matmul(
                        p2[:, c, :], lhsT=ht[1][:, c * 128:(c + 1) * 128],
                        rhs=w2k[1][:], start=False, stop=not add_b2,
                    )
                    if add_b2:
                        nc.tensor.matmul(
                            p2[:, c, :], lhsT=ones_row[:],
                            rhs=b2q[:, c * h:(c + 1) * h],
                            start=False, stop=True,
                        )
                # norms^2: ACT squares psum into bf16; reduce on Pool while
                # it is idle (before the gathers start), on DVE afterwards
                sqb = gp.tile([128, nch, h], BF16, tag="sqb")
                nc.scalar.activation(sqb[:], p2[:], AF.Square)
                n2 = sp.tile([128, nch], F32, tag="n2")
                nc.vector.tensor_reduce(
                    out=n2[:], in_=sqb[:], axis=AX.X, op=OP.add)
                # max(sqrt(n2), eps) == sqrt(n2 + eps^2) here (exact for
                # n2=0 and for any real row)
                s_ = sp.tile([128, nch], F32, tag="s")
                nc.scalar.activation(s_[:], n2[:], AF.Sqrt, bias=eps2[:])
                invb = sp.tile([128, nch], BF16, tag="invb")
                with nc.allow_low_precision(reason="bf16 inv-norm scale"):
                    nc.vector.reciprocal(invb[:], s_[:])
                gnb = gp.tile([128, nch, h], table_dt, tag="gnb")
                nc.vector.tensor_tensor(
                    out=gnb[:], in0=p2[:],
                    in1=invb[:].to_broadcast([128, nch, h]), op=OP.mult,
                )
                if b < blo:
                    dst = gn_lo[n0:n0 + f, :]
                else:
                    dst = gn_hi[n0 - n_lo:n0 - n_lo + f, :]
                wi = nc.sync.dma_start(
                    out=dst.rearrange("(c p) h -> p c h", p=128),
                    in_=gnb[:],
                )
                (lo_writes if b < blo else hi_writes).append(wi)

                if b == blo - 1 and n_blk > blo:
                    for i, (g_, c0_, nI_) in enumerate(g0_chunks):
                        gathered[(g_, c0_)] = emit_gathers(
                            g_, c0_, nI_, list(lo_writes) if i == 0 else [])
                for g_, c0_, nI_ in compute_at[b]:
                    emit_compute(g_, c0_, nI_, *gathered[(g_, c0_)])
                    done_compute.add((g_, c0_))

            # remaining gathers (Pool-serial; WAR on the tile rings paces
            # them against the computes), then remaining computes in order
            rest_computes = [c for c in g0_chunks
                             if (c[0], c[1]) not in done_compute]
            if n_blk <= blo:
                for i, (g_, c0_, nI_) in enumerate(g0_chunks):
                    gathered[(g_, c0_)] = emit_gathers(
                        g_, c0_, nI_, list(lo_writes) if i == 0 else [])
            for g in (1, 2, 3):
                for i, (g_, c0_, nI_) in enumerate(chunk_list(g)):
                    deps = []
                    if g == 1 and i == 0:
                        deps = list(hi_writes)
                        if n_blk <= blo:
                            deps += list(lo_writes)
                    gathered[(g_, c0_)] = emit_gathers(g_, c0_, nI_, deps)
                    rest_computes.append((g_, c0_, nI_))
            for g_, c0_, nI_ in rest_computes:
                emit_compute(g_, c0_, nI_, *gathered[(g_, c0_)])
            nc.sync.dma_start(out=dots_out[:], in_=dots[:])

    return nc


def make_inputs(emb, W1, b1, W2, b2, col, row, n_pad, gcaps, ncores):
    """Host-side prep: transposes, bf16 rounding, per-core group shards.

    Returns (in_maps, scatter) where scatter[c] = (positions, lens)
    for reassembling per-core outputs.
    """
    h = emb.shape[1]
    embT = np.zeros((h, n_pad), dtype=ml_dtypes.bfloat16)
    embT[:, :emb.shape[0]] = emb.astype(ml_dtypes.bfloat16).T
    w1t = np.ascontiguousarray(W1.astype(ml_dtypes.bfloat16).T)
    w2t = np.ascontiguousarray(W2.astype(ml_dtypes.bfloat16).T)
    b1c = np.ascontiguousarray(b1.astype(np.float32).reshape(h, 1))
    b2quad = np.tile(b2.astype(ml_dtypes.bfloat16).reshape(1, h), (1, 4))
    epc = len(col) // ncores
    goffs = [int(x) for x in np.cumsum([0] + list(gcaps[:-1]))]
    tote = sum(gcaps)

    def wrap16(a):
        return np.tile(a.reshape(-1, 16).T, (8, 1)).astype(np.int16)

    in_maps = []
    scatter = []
    for c in range(ncores):
        cs = col[c * epc:(c + 1) * epc].astype(np.int64)
        rs = row[c * epc:(c + 1) * epc].astype(np.int64)
        gid = (cs >= HALF) * 2 + (rs >= HALF)
        # padded tail slots get -1: dma_gather trims trailing negatives, so
        # pad slots cost no descriptor generation and no transfer
        colw = np.zeros(tote, dtype=np.int16)
        roww = np.zeros(tote, dtype=np.int16)
        positions = []
        lens = []
        for g in range(4):
            pos = np.nonzero(gid == g)[0]
            pos = pos[np.argsort(cs[pos], kind="stable")]
            ng = len(pos)
            assert ng <= gcaps[g], f"group {g} overflow: {ng} > {gcaps[g]}"
            cb = HALF if g >= 2 else 0
            rb = HALF if g % 2 else 0
            colw[goffs[g]:goffs[g] + ng] = (cs[pos] - cb).astype(np.int16)
            roww[goffs[g]:goffs[g] + ng] = (rs[pos] - rb).astype(np.int16)
            positions.append(pos)
            lens.append(ng)
        in_maps.append({
            "embT": embT, "w1t": w1t, "w2t": w2t, "b1c": b1c,
            "b2quad": b2quad,
            "colw": wrap16(colw), "roww": wrap16(roww),
        })
        scatter.append((positions, lens))
    return in_maps, scatter


def unshard_output(outs, scatter, gcaps, epc, ncores):
    goffs = [int(x) for x in np.cumsum([0] + list(gcaps[:-1]))]
    parts = []
    for c in range(ncores):
        dots = np.asarray(outs[c]["dots"]).T.reshape(-1)
        positions, lens = scatter[c]
        res = np.empty(epc, dtype=np.float32)
        for g in range(4):
            res[positions[g]] = dots[goffs[g]:goffs[g] + lens[g]]
        parts.append(res)
    return np.concatenate(parts)


_NC_CACHE = {}


def get_nc(add_b2=False):
    key = ("nc", bool(add_b2))
    if key not in _NC_CACHE:
        nc_ = build_bass(NPAD, NBLK, F, GCAPS, HALF, GCHUNK, add_b2=add_b2)
        nc_.compile()
        _NC_CACHE[key] = nc_
    return _NC_CACHE[key]


def kernel(emb, edge_index, W1, b1, W2, b2):
    emb = np.asarray(emb)
    edge_index = np.asarray(edge_index)
    W1, b1, W2, b2 = (np.asarray(a) for a in (W1, b1, W2, b2))
    col = edge_index[0].astype(np.int64)
    row = edge_index[1].astype(np.int64)

    nc = get_nc(add_b2=bool(np.any(b2)))
    in_maps, scatter = make_inputs(emb, W1, b1, W2, b2, col, row, NPAD, GCAPS, NCORES)
    # drop inputs the chosen program does not declare
    declared = {a.memorylocations[0].name
                for a_ in [nc.m.functions[0].allocations] for a in a_
                if isinstance(a, mybir.MemoryLocationSet)
                and a.kind == "ExternalInput"}
    in_maps = [{k: v for k, v in m.items() if k in declared} for m in in_maps]
    res = run_bass_kernel_spmd(nc, in_maps, core_ids=list(range(NCORES)))
    return unshard_output(res.results, scatter, GCAPS, EPC, NCORES).astype(np.float32)
